# revision 27
# baseline (speedup 1.0000x reference)
"""Deep Kalman Filter (nn_DeepKalmanFilter) Trainium2 Bass kernel.

Strategy
--------
Data-parallel over batch: B=512 is sharded 8 ways (64 per core); all
parameters are replicated. Per core the computation is restructured as:

1. A 256-step sequential scan that carries ONLY the minimal recurrent
   state. All activation functions come from the single `natural_log_exp`
   ACT table set (exp/ln); tanh is computed as 1-2*sigma(-2x) with
   sigma(-2x) = 1/(1+exp(2x)) via ACT Exp + DVE reciprocal_approx_fast,
   and the affine 1-2*sigma is folded into downstream matmul weights on
   the host. softplus(s) = Ln(Exp(s)+1) (both exact ACT functions).

   Per step the loop-carried chain is:
     exp(2*pre) [ACT] -> +1, recip, qh [DVE] -> s-matmuls [PE]
     -> exp(s), ln(.+1) [ACT] -> spe = sp*eps [DVE] -> pre-matmuls [PE]

   where qh = 0.5 - c + h~  (h~ = 0.5*h of the Elman RNN, folded), and
   pre_{t+1} = (Wzh@Wloc)@(h~ - c) + Wzh@spe + const   (host-composed).

2. Everything else (q_loc, z, emitter MLP, gated transition) depends only
   on the saved per-step values (qh, sp, eps) and is executed as large
   batched matmuls/elementwise in chunks of 8 steps (free dim 512),
   interleaved with the scan so it hides in engine idle time.

Outputs are produced in [feature, t, batch] device layout and transposed
to [B, T, 288] on the host (host work is not part of HW exec time).
"""
import sys

if "/opt/trn_rl_repo" not in sys.path:
    sys.path.insert(0, "/opt/trn_rl_repo")

import numpy as np

import concourse.bass as bass
import concourse.bacc as bacc
import concourse.mybir as mybir
import concourse.tile as tile
from concourse.bass_utils import run_bass_kernel_spmd

# ---------------------------------------------------------------------------
# The act-table-load inserter assigns each ActivationFunctionType the FIRST
# table set containing it (Exp -> exp_and_others, Ln -> natural_log), which
# makes an Exp;Ln chain reload tables on every instruction (~2.7us each).
# Everything this kernel runs on ScalarE (Exp, Ln, Relu, Identity) lives in
# the single `natural_log_exp_and_others` set, so hide those functions from
# every other set (keys and order preserved -- set ids must stay stable).
# ---------------------------------------------------------------------------
import concourse.hw_specs as _hw_specs

_orig_gat = _hw_specs.get_activation_tables
_COMBINED = "natural_log_exp_and_others"


def _patched_gat(arch):
    tabs = _orig_gat(arch)
    if _COMBINED not in tabs:
        return tabs
    comb = tabs[_COMBINED]
    return {name: (funcs if name == _COMBINED else funcs - comb)
            for name, funcs in tabs.items()}


_hw_specs.get_activation_tables = _patched_gat
bacc.get_activation_tables = _patched_gat

AF = mybir.ActivationFunctionType
ALU = mybir.AluOpType
F32 = mybir.dt.float32
F32R = mybir.dt.float32r


def _r(ap):
    """Reinterpret an fp32 AP as float32r for the PE fast fp32 path."""
    return ap.bitcast(F32R)

B, T, D_IN, Z, E_DIM, TR_DIM, R_DIM = 512, 256, 32, 64, 256, 256, 128
N_CORES = 8
BL = B // N_CORES          # 64 batch per core
TB = T * BL                # positions per core
CH = 8                     # scan steps per epilogue chunk
FD = CH * BL               # 512 free-dim per chunk
N_CHUNK = T // CH

TRACE = [False]            # test.py can enable profiling


# --------------------------------------------------------------------------
# host-side parameter preparation
# --------------------------------------------------------------------------
def _prep_params(p):
    f = lambda a: np.ascontiguousarray(np.asarray(a, np.float32))
    W_ih, b_ih = f(p["W_ih"]), f(p["b_ih"])
    W_hh, b_hh = f(p["W_hh"]), f(p["b_hh"])
    z_q0 = f(p["z_q0"])
    Wc_zh, bc_zh = f(p["Wc_zh"]), f(p["bc_zh"])
    Wc_loc, bc_loc = f(p["Wc_loc"]), f(p["bc_loc"])
    Wc_sc, bc_sc = f(p["Wc_sc"]), f(p["bc_sc"])
    M = Wc_zh @ Wc_loc                       # [128,128]

    def col(v):
        return np.ascontiguousarray(v.reshape(-1, 1).astype(np.float32))

    def chunks2(v):  # [256] -> [128, 2]
        return np.ascontiguousarray(np.stack([v[:128], v[128:]], axis=1))

    def packT2(W):   # [M_out, 256] -> W.T k-chunks packed [128, 2*M_out]
        WT = W.T     # [256, M_out]
        return np.ascontiguousarray(np.concatenate([WT[:128], WT[128:]], axis=1))

    d = {
        # scan
        "ihT": np.ascontiguousarray((0.5 * W_ih).T),          # [32,128]
        "hhT": np.ascontiguousarray(W_hh.T),                  # [128,128]
        "bh": col(0.5 * (b_ih + b_hh)),                       # [128,1]
        "schT": np.ascontiguousarray(Wc_sc.T),                # [128,64]
        "sccT": np.ascontiguousarray((-Wc_sc).T),             # [128,64]
        "bs": col(0.5 * Wc_sc.sum(1) + bc_sc),                # [64,1]
        "preqT": np.ascontiguousarray(M.T),                   # [128,128]
        "prespeT": np.ascontiguousarray(Wc_zh.T),             # [64,128]
        "bpre2": col(2.0 * (Wc_zh @ bc_loc + bc_zh)),
        "bpre0": col(2.0 * (Wc_zh @ z_q0 + bc_zh)),           # t=0 bias
        # epilogue
        "locT": np.ascontiguousarray(Wc_loc.T),               # [128,64]
        "bloc": col(bc_loc),
        "e1T": np.ascontiguousarray(f(p["We1"]).T),           # [64,256]
        "be1": chunks2(f(p["be1"])),
        "e2T": packT2(f(p["We2"])),                           # [128,512]
        "be2": chunks2(f(p["be2"])),
        "e3T": packT2(f(p["We3"])),                           # [128,64]
        "be3": col(f(p["be3"])),
        "g1T": np.ascontiguousarray(f(p["Wt_g1"]).T),         # [64,256]
        "bg1": chunks2(f(p["bt_g1"])),
        "g2T": packT2(f(p["Wt_g2"])),                         # [128,128]
        "nbg2": col(-f(p["bt_g2"])),
        "p1T": np.ascontiguousarray(f(p["Wt_p1"]).T),         # [64,256]
        "bp1": chunks2(f(p["bt_p1"])),
        "p2T": packT2(f(p["Wt_p2"])),                         # [128,128]
        "zlT": np.ascontiguousarray(f(p["Wt_zl"]).T),         # [64,64]
        "nzlT": np.ascontiguousarray((-f(p["Wt_zl"])).T),     # [64,64]
        "bzl": col(f(p["bt_zl"])),
        "bD": col(f(p["bt_p2"]) - f(p["bt_zl"])),
        "sigT": np.ascontiguousarray(f(p["Wt_sig"]).T),       # [64,64]
        "bsig": col(f(p["bt_sig"])),
        "z0t": np.ascontiguousarray(np.broadcast_to(z_q0[:, None], (Z, BL)).copy()),
    }
    return d


# --------------------------------------------------------------------------
# device program (identical on all 8 cores)
# --------------------------------------------------------------------------
def _build(wspec, reps=1):
    nc = bacc.Bacc("TRN2", target_bir_lowering=False, debug=False,
                   enable_asserts=False, num_devices=N_CORES)

    seq_d = nc.dram_tensor("SEQ", [T * D_IN, BL], F32, kind="ExternalInput")
    eps_d = nc.dram_tensor("EPS", [T * Z, BL], F32, kind="ExternalInput")
    _R_WEIGHTS = {"e1T", "e2T", "e3T", "g1T", "g2T", "p1T", "p2T",
                  "zlT", "nzlT", "sigT"}
    w_d = {k: nc.dram_tensor(k, list(v.shape),
                             F32R if k in _R_WEIGHTS else F32,
                             kind="ExternalInput")
           for k, v in wspec.items()}
    mu_d = nc.dram_tensor("MU", [D_IN, TB], F32, kind="ExternalOutput")
    qloc_d = nc.dram_tensor("QLOC", [Z, TB], F32, kind="ExternalOutput")
    qsc_d = nc.dram_tensor("QSC", [Z, TB], F32, kind="ExternalOutput")
    ploc_d = nc.dram_tensor("PLOC", [Z, TB], F32, kind="ExternalOutput")
    psc_d = nc.dram_tensor("PSC", [Z, TB], F32, kind="ExternalOutput")

    seq_ap = seq_d.ap().rearrange("(t d) b -> d t b", d=D_IN)   # [32,T,BL]
    eps_ap = eps_d.ap().rearrange("(t z) b -> z t b", z=Z)      # [64,T,BL]

    with tile.TileContext(nc) as tc:
        _emit(nc, tc, w_d, seq_ap, eps_ap, mu_d, qloc_d, qsc_d, ploc_d, psc_d,
              reps=reps)
    nc.finalize()
    return nc


def _emit(nc, tc, w_d, seq_ap, eps_ap, mu_d, qloc_d, qsc_d, ploc_d, psc_d, reps=1):
    import contextlib
    ctx = contextlib.ExitStack()
    with ctx:
        const = ctx.enter_context(tc.tile_pool(name="const", bufs=1))
        big = ctx.enter_context(tc.tile_pool(name="big", bufs=1))
        sring = ctx.enter_context(tc.tile_pool(name="sring", bufs=8))
        ioring = ctx.enter_context(tc.tile_pool(name="ioring", bufs=3))
        epool = ctx.enter_context(tc.tile_pool(name="epool", bufs=2))
        ps_scan = ctx.enter_context(tc.tile_pool(name="ps_scan", bufs=2, space="PSUM"))
        ps_scan1 = ctx.enter_context(tc.tile_pool(name="ps_scan1", bufs=1, space="PSUM"))
        ps_epi = ctx.enter_context(tc.tile_pool(name="ps_epi", bufs=4, space="PSUM"))

        # ---- load weights/biases into SBUF once
        w = {}
        for k, d in w_d.items():
            shp = list(d.shape)
            wt = const.tile(shp, d.dtype, tag=f"w_{k}")
            nc.sync.dma_start(wt[:], d.ap())
            w[k] = wt

        zero_sb = const.tile([R_DIM, BL], F32, tag="zero")
        nc.vector.memset(zero_sb[:], 0.0)
        prev_tail = const.tile([Z, BL], F32, tag="ptail")
        nc.sync.dma_start(prev_tail[:], w_d["z0t"].ap())

        qh_buf = big.tile([R_DIM, TB], F32, tag="qh")           # 64KB/part

        for _rep in range(reps):
            if _rep > 0:
                nc.sync.dma_start(prev_tail[:], w_d["z0t"].ap())
            # ---- scan + interleaved epilogue ----
            from collections import deque
            pending = deque()
            h_prev = None
            pp_prev = None
            x_tile = e_tile = qsc_tile = None

            for t in range(T):
                ri = t % CH
                ci = t // CH
                if ri == 0:
                    x_tile = ioring.tile([D_IN, FD], F32, tag="xin")
                    nc.sync.dma_start(
                        x_tile[:].rearrange("d (t b) -> d t b", t=CH),
                        seq_ap[:, t:t + CH, :])
                    e_tile = ioring.tile([Z, FD], F32, tag="ein")
                    nc.sync.dma_start(
                        e_tile[:].rearrange("z (t b) -> z t b", t=CH),
                        eps_ap[:, t:t + CH, :])
                    qsc_tile = ioring.tile([Z, FD], F32, tag="qscr")

                sl = slice(ri * BL, (ri + 1) * BL)

                # h-chain: h~ = relu(0.5*Wih@x + Whh@h~ + bh)
                ph = ps_scan1.tile([R_DIM, BL], F32, tag="ph")
                if t == 0:
                    nc.tensor.matmul(ph[:], w["ihT"][:], x_tile[:, sl],
                                     start=True, stop=True)
                else:
                    nc.tensor.matmul(ph[:], w["ihT"][:], x_tile[:, sl],
                                     start=True, stop=False)
                    nc.tensor.matmul(ph[:], w["hhT"][:], h_prev[:],
                                     start=False, stop=True)
                h_cur = sring.tile([R_DIM, BL], F32, tag="h")
                nc.vector.tensor_scalar(h_cur[:], ph[:], w["bh"][:], 0.0,
                                        ALU.add, ALU.max)

                # z-chain: c = 1/(1+exp(2*pre))
                u1 = sring.tile([R_DIM, BL], F32, tag="u1")
                if t == 0:
                    nc.scalar.activation(u1[:], zero_sb[:], AF.Exp,
                                         bias=w["bpre0"][:], scale=2.0)
                else:
                    nc.scalar.activation(u1[:], pp_prev[:], AF.Exp,
                                         bias=w["bpre2"][:], scale=2.0)
                v1 = sring.tile([R_DIM, BL], F32, tag="v1")
                nc.vector.tensor_scalar_add(v1[:], u1[:], 1.0)
                c = sring.tile([R_DIM, BL], F32, tag="c")
                nc.vector.reciprocal_approx_fast(c[:], v1[:])
                # qh = (c * -1 + 0.5) + h~   (= q_h; feeds s/pre matmuls + epilogue)
                qh_sl = qh_buf[:, t * BL:(t + 1) * BL]
                nc.vector.affine_then_add(qh_sl, c[:], h_cur[:], scale=-1.0, bias=0.5)

                # s = Wsc@h~ - Wsc@c (+bias in Exp);  sp = ln(exp(s)+1)
                pss = ps_scan1.tile([Z, BL], F32, tag="ps")
                nc.tensor.matmul(pss[:], w["schT"][:], h_cur[:], start=True, stop=False)
                nc.tensor.matmul(pss[:], w["sccT"][:], c[:], start=False, stop=True)
                u2 = sring.tile([Z, BL], F32, tag="u2")
                nc.scalar.activation(u2[:], pss[:], AF.Exp, bias=w["bs"][:], scale=1.0)
                sp_sl = qsc_tile[:, sl]
                nc.scalar.activation(sp_sl, u2[:], AF.Ln, bias=1.0, scale=1.0)

                # spe = sp * eps_t  (ring; also recomputed batched in epilogue)
                spe = sring.tile([Z, BL], F32, tag="spe")
                nc.vector.tensor_mul(spe[:], sp_sl, e_tile[:, sl])

                # pre_{t+1} = (Wzh@Wloc)@qh + Wzh@spe (+bias in next Exp)
                pp = ps_scan.tile([R_DIM, BL], F32, tag="pp")
                nc.tensor.matmul(pp[:], w["preqT"][:], qh_sl, start=True, stop=False)
                nc.tensor.matmul(pp[:], w["prespeT"][:], spe[:], start=False, stop=True)

                h_prev, pp_prev = h_cur, pp

                if ri == CH - 1:
                    nc.sync.dma_start(qsc_d.ap()[:, ci * FD:(ci + 1) * FD], qsc_tile[:])
                    pending.append(_emit_chunk(
                        nc, tc, w, epool, ps_epi, qh_buf, prev_tail,
                        (qsc_tile, e_tile), ci, mu_d, qloc_d, ploc_d, psc_d))
                # advance the (one-chunk-delayed) epilogue, two slices per step
                for _ in range(2):
                    if pending:
                        try:
                            next(pending[0])
                        except StopIteration:
                            pending.popleft()

            while pending:
                for _ in pending.popleft():
                    pass


def _emit_chunk(nc, tc, w, epool, ps_epi, qh_buf, prev_tail, refs, ci,
                mu_d, qloc_d, ploc_d, psc_d):
    """Batched work for scan steps [ci*CH, (ci+1)*CH): q_loc, z, emitter,
    gated transition. Free dim FD=512. Generator: yields at slice
    boundaries so emission interleaves with later scan steps (the Tile
    scheduler's priority is emission order; fine slices let this bulk
    work fill the scan chain's engine idle gaps instead of blocking it)."""
    sp_tile, e_tile = refs
    j0 = ci * FD
    qh_c = qh_buf[:, j0:j0 + FD]
    osl = slice(j0, j0 + FD)

    # q_loc = Wloc@qh + bloc
    pql = ps_epi.tile([Z, FD], F32, tag="eps1")
    nc.tensor.matmul(pql[:], w["locT"][:], qh_c, start=True, stop=True)
    qloc_sb = epool.tile([Z, FD], F32, tag="qloc")
    nc.scalar.activation(qloc_sb[:], pql[:], AF.Identity, bias=w["bloc"][:])
    nc.sync.dma_start(qloc_d.ap()[:, osl], qloc_sb[:])

    # z = q_loc + sp*eps ; z_ext = [prev_tail | z] so zp = shift-by-one-step
    spe_b = epool.tile([Z, FD], F32, tag="speb")
    nc.vector.tensor_mul(spe_b[:], sp_tile[:], e_tile[:])
    yield
    z_ext = epool.tile([Z, FD + BL], F32R, tag="zext")
    nc.gpsimd.tensor_copy(z_ext[:, 0:BL], prev_tail[:])
    nc.vector.tensor_add(z_ext[:, BL:BL + FD], qloc_sb[:], spe_b[:])
    nc.gpsimd.tensor_copy(prev_tail[:], z_ext[:, FD:FD + BL])
    zz = z_ext[:, BL:BL + FD]
    zp = z_ext[:, 0:FD]
    yield

    # ---- emitter: mu = We3@relu(We2@relu(We1@z+b1)+b2)+b3
    e1s = []
    for m in range(2):
        pe = ps_epi.tile([128, FD], F32, tag="eps1")
        nc.tensor.matmul(pe[:], w["e1T"][:, m * 128:(m + 1) * 128], zz,
                         start=True, stop=True)
        e1 = epool.tile([128, FD], F32R, tag=f"e1_{m}")
        nc.scalar.activation(e1[:], pe[:], AF.Relu, bias=w["be1"][:, m:m + 1])
        e1s.append(e1)
        yield
    e2s = []
    for m in range(2):
        pe = ps_epi.tile([128, FD], F32, tag="eps1")
        nc.tensor.matmul(pe[:], w["e2T"][:, m * 128:m * 128 + 128], e1s[0][:],
                         start=True, stop=False)
        nc.tensor.matmul(pe[:], w["e2T"][:, 256 + m * 128:256 + m * 128 + 128], e1s[1][:], start=False, stop=True)
        e2 = epool.tile([128, FD], F32R, tag=f"e2_{m}")
        nc.scalar.activation(e2[:], pe[:], AF.Relu, bias=w["be2"][:, m:m + 1])
        e2s.append(e2)
        yield
    pmu = ps_epi.tile([D_IN, FD], F32, tag="eps1")
    nc.tensor.matmul(pmu[:], w["e3T"][:, 0:D_IN], e2s[0][:], start=True, stop=False)
    nc.tensor.matmul(pmu[:], w["e3T"][:, D_IN:2 * D_IN], e2s[1][:],
                     start=False, stop=True)
    mu_sb = epool.tile([D_IN, FD], F32, tag="mu")
    nc.scalar.activation(mu_sb[:], pmu[:], AF.Identity, bias=w["be3"][:])
    nc.sync.dma_start(mu_d.ap()[:, osl], mu_sb[:])
    yield

    # ---- transition (inputs zp): gate, prop, lin
    g1s = []
    for m in range(2):
        pg = ps_epi.tile([128, FD], F32, tag="eps1")
        nc.tensor.matmul(pg[:], w["g1T"][:, m * 128:(m + 1) * 128], zp,
                         start=True, stop=True)
        g1 = epool.tile([128, FD], F32R, tag=f"g1_{m}")
        nc.scalar.activation(g1[:], pg[:], AF.Relu, bias=w["bg1"][:, m:m + 1])
        g1s.append(g1)
        yield
    pgt = ps_epi.tile([Z, FD], F32, tag="eps1")
    nc.tensor.matmul(pgt[:], w["g2T"][:, 0:Z], g1s[0][:], start=True, stop=False)
    nc.tensor.matmul(pgt[:], w["g2T"][:, Z:2 * Z], g1s[1][:], start=False, stop=True)
    eg = epool.tile([Z, FD], F32, tag="eg")
    nc.scalar.activation(eg[:], pgt[:], AF.Exp, bias=w["nbg2"][:], scale=-1.0)
    vg = epool.tile([Z, FD], F32, tag="vg")
    nc.gpsimd.tensor_scalar_add(vg[:], eg[:], 1.0)
    gate = epool.tile([Z, FD], F32, tag="gate")
    nc.vector.reciprocal_approx_fast(gate[:], vg[:])
    yield

    phs = []
    for m in range(2):
        pg = ps_epi.tile([128, FD], F32, tag="eps1")
        nc.tensor.matmul(pg[:], w["p1T"][:, m * 128:(m + 1) * 128], zp,
                         start=True, stop=True)
        p1 = epool.tile([128, FD], F32R, tag=f"ph_{m}")
        nc.scalar.activation(p1[:], pg[:], AF.Relu, bias=w["bp1"][:, m:m + 1])
        phs.append(p1)
        yield

    plin = ps_epi.tile([Z, FD], F32, tag="eps1")
    nc.tensor.matmul(plin[:], w["zlT"][:], zp, start=True, stop=True)
    linb = epool.tile([Z, FD], F32, tag="linb")
    nc.scalar.activation(linb[:], plin[:], AF.Identity, bias=w["bzl"][:])

    # D = prop - lin (bias folded): p_loc = lin + gate*D ; prop = D + lin
    pD = ps_epi.tile([Z, FD], F32, tag="eps1")
    nc.tensor.matmul(pD[:], w["p2T"][:, 0:Z], phs[0][:], start=True, stop=False)
    nc.tensor.matmul(pD[:], w["p2T"][:, Z:2 * Z], phs[1][:], start=False, stop=False)
    nc.tensor.matmul(pD[:], w["nzlT"][:], zp, start=False, stop=True)
    Db = epool.tile([Z, FD], F32, tag="Db")
    nc.scalar.activation(Db[:], pD[:], AF.Identity, bias=w["bD"][:])
    yield

    yield
    t1 = epool.tile([Z, FD], F32, tag="t1")
    nc.vector.tensor_mul(t1[:], gate[:], Db[:])
    ploc_sb = epool.tile([Z, FD], F32, tag="ploc")
    nc.vector.tensor_add(ploc_sb[:], t1[:], linb[:])
    nc.sync.dma_start(ploc_d.ap()[:, osl], ploc_sb[:])

    yield
    propb = epool.tile([Z, FD], F32, tag="propb")
    nc.vector.tensor_add(propb[:], Db[:], linb[:])
    rp = epool.tile([Z, FD], F32R, tag="rp")
    nc.gpsimd.tensor_scalar_max(rp[:], propb[:], 0.0)

    psg = ps_epi.tile([Z, FD], F32, tag="eps1")
    nc.tensor.matmul(psg[:], w["sigT"][:], rp[:], start=True, stop=True)
    esg = epool.tile([Z, FD], F32, tag="esg")
    nc.scalar.activation(esg[:], psg[:], AF.Exp, bias=w["bsig"][:])
    yield
    psc_sb = epool.tile([Z, FD], F32, tag="pscsb")
    nc.scalar.activation(psc_sb[:], esg[:], AF.Ln, bias=1.0)
    nc.sync.dma_start(psc_d.ap()[:, osl], psc_sb[:])


# --------------------------------------------------------------------------
# public entry point
# --------------------------------------------------------------------------
def kernel(sequence, eps, params):
    sequence = np.asarray(sequence, np.float32)
    eps = np.asarray(eps, np.float32)
    wd = _prep_params(params)

    # device layouts: [T, feat, B] per core shard
    seq_t = np.ascontiguousarray(sequence.transpose(1, 2, 0))   # [T, D, B]
    eps_t = np.ascontiguousarray(eps.transpose(1, 2, 0))        # [T, Z, B]

    nc = _build(wd)

    in_maps = []
    for i in range(N_CORES):
        sh = slice(i * BL, (i + 1) * BL)
        m = {k: v for k, v in wd.items()}
        m["SEQ"] = np.ascontiguousarray(seq_t[:, :, sh]).reshape(T * D_IN, BL)
        m["EPS"] = np.ascontiguousarray(eps_t[:, :, sh]).reshape(T * Z, BL)
        in_maps.append(m)

    trace = TRACE[0]
    try:
        res = run_bass_kernel_spmd(nc, in_maps, core_ids=list(range(N_CORES)),
                                   trace=trace)
    except ModuleNotFoundError:
        # NTFF profiling hook unavailable under this axon client
        trace = False
        res = run_bass_kernel_spmd(nc, in_maps, core_ids=list(range(N_CORES)))
    if res.exec_time_ns is not None:
        print(f"HW exec time: {res.exec_time_ns} ns")
    elif TRACE[0]:
        # No on-device profiling path in this environment: report the
        # cost-model (TimelineSim) predicted device time instead.
        try:
            from concourse.timeline_sim import TimelineSim
            pred = TimelineSim(nc).simulate()
            print(f"HW exec time: {pred:.0f} ns (TimelineSim cost-model "
                  f"prediction; NTFF profiling unavailable)")
        except Exception as e:
            print(f"HW exec time: unavailable ({e})")

    out = np.empty((B, T, D_IN + 4 * Z), np.float32)
    for i in range(N_CORES):
        r = res.results[i]
        blk = np.concatenate([
            r["MU"].reshape(D_IN, T, BL),
            r["QLOC"].reshape(Z, T, BL),
            r["QSC"].reshape(Z, T, BL),
            r["PLOC"].reshape(Z, T, BL),
            r["PSC"].reshape(Z, T, BL),
        ], axis=0)                                   # [288, T, BL]
        out[i * BL:(i + 1) * BL] = blk.transpose(2, 1, 0)
    return out

